# revision 75
# baseline (speedup 1.0000x reference)
"""Trainium2 Bass kernel for CustomBertAttention (B=4, S=2048, H=1024, NH=16).

Sharding: 8 cores = (batch b in 0..3) x (query-half j in 0..1).
Each core computes attention for NQ=1024 query rows of one batch against the
full NKV=2048-key sequence of that batch.  The per-core KV sequence is
permuted host-side so the core's own query rows come FIRST (one SPMD program
for all cores); bias-matrix columns are permuted identically.

Precision/layout strategy (vs the straightforward version):
  * All projection operands (X^T, Wq/Wk/Wv/Wo^T) are packed HOST-SIDE into
    fp8e4m3 DoubleRow layout [64, c, 2, cols] (hidden h = 128c + 64i + p) so
    every projection matmul runs at 0.5 cycles/row -- and no on-device
    transposes or casts are needed at all.  Weights are pre-scaled by 2^6
    host-side (fp8e4m3 subnormal cutoff) and the 2^-6 is folded into the
    PSUM evacuation.
  * Scores/ctx stay bf16: K^T/Q^T per head-pair row-packed (2 heads at
    partitions 0:64 / 64:128), transposed scores S^T[k,q] per (z, qc-chunk)
    in one PSUM bank, exp on ScalarE (scale=1/8) into a per-kt es tile,
    multiplied by expB = exp(coef*B^T) (bf16; built once with the coef
    folded into the activation scale operand).
  * Denominators via the ones-augmented V' stationary (row 64 of the ctx
    PSUM); 1/denom computed by DVE reciprocal straight from PSUM, replicated
    across partitions with a ones-matmul, and applied IN the ctx evacuation
    multiply, which writes ctx^T directly in fp8 DoubleRow layout for the
    output projection.
  * Engine budget: ScalarE does only exp (+expB+LN tail), DVE does PSUM
    evacuations + most es*expB multiplies, GpSimd (Pool) takes a slice of
    the es multiplies (SBUF-only), PE does matmuls only.
"""

from collections import deque
from contextlib import ExitStack

import numpy as np
import ml_dtypes

import concourse.bass as bass
import concourse.mybir as mybir
import concourse.tile as tile
from concourse.bass_utils import run_bass_kernel_spmd
from concourse.masks import make_identity

F32 = mybir.dt.float32
BF16 = mybir.dt.bfloat16
FP8 = mybir.dt.float8e4
AF = mybir.ActivationFunctionType
AX = mybir.AxisListType
ALU = mybir.AluOpType
DR = mybir.MatmulPerfMode.DoubleRow

P = 128
EPS = 1e-12
I32 = mybir.dt.int32
FEXP_A = float(2 ** 23 / np.log(2)) / 8.0   # fast-exp scale (folds 1/8)
FEXP_B = float(127 * 2 ** 23 - 366000)      # fast-exp bias (error centering)
WSC = 64.0      # host-side weight pre-scale (2^6), undone in evacuations
IWSC = 1.0 / WSC


def split_multi_waits(nc):
    """Pinned walrus supports only ONE sync-wait per instruction; split extras
    onto preceding same-engine NoOps."""
    n_split = 0
    for fn in nc.m.functions:
        for blk in fn.blocks:
            new_insts = []
            for inst in blk.instructions:
                si = inst.sync_info
                if si is not None and si.on_wait and len(si.on_wait) > 1:
                    waits = list(si.on_wait)
                    for w in waits[:-1]:
                        nop = mybir.InstNoOp(
                            name=f"{inst.name}-wsplit{n_split}",
                            engine=inst.engine,
                        )
                        nop.sync_info = mybir.SyncInfo(on_wait=[w], on_update=[])
                        new_insts.append(nop)
                        n_split += 1
                    inst.sync_info = mybir.SyncInfo(
                        on_wait=[waits[-1]], on_update=list(si.on_update)
                    )
                new_insts.append(inst)
            blk.instructions = new_insts
    return n_split


def build_program(NKV=2048, NQ=1024, H=1024, NH=16, split=True):
    HD = H // NH
    assert HD == 64
    KT = NKV // P           # key seq tiles (16)
    HOT = H // P            # 128-wide hidden chunks (8) = head pairs
    QTW = 512               # q span per score matmul
    NQC = NQ // QTW         # 2
    NPAIR = NH // 2         # 8
    VW = HD + 1             # V' width per head (64 + ones col)
    CW = 512
    NHC = H // CW

    nc = bass.Bass("TRN2", target_bir_lowering=False, debug=False)

    # fp8 DoubleRow-packed operands: [64, c, i, cols] flat as [64, HOT*2*cols]
    xTd_d = nc.dram_tensor("xTd", [64, HOT * 2 * NKV], FP8, kind="ExternalInput").ap()
    w_d = {}
    for wname in ("wk", "wq", "wv", "wo"):
        w_d[wname] = nc.dram_tensor(wname, [64, HOT * 2 * H], FP8, kind="ExternalInput").ap()
    xq = nc.dram_tensor("xq", [NQ, H], BF16, kind="ExternalInput").ap()
    biasT = nc.dram_tensor("biasT", [NKV, NQ], F32, kind="ExternalInput").ap()
    bs = {}
    for bname in ("bq", "bk", "bv", "bo"):
        bs[bname] = nc.dram_tensor(bname, [H], F32, kind="ExternalInput").ap()
    gamma = nc.dram_tensor("gamma", [H], F32, kind="ExternalInput").ap()
    beta = nc.dram_tensor("beta", [H], F32, kind="ExternalInput").ap()
    coef = nc.dram_tensor("coef", [1], F32, kind="ExternalInput").ap()
    out = nc.dram_tensor("out", [NQ, H], F32, kind="ExternalOutput").ap()

    with tile.TileContext(nc) as tc, ExitStack() as top:
        pers = top.enter_context(tc.tile_pool(name="pers", bufs=1))
        expB = pers.tile([P, KT, NQ], BF16, tag="expB")
        ctxd = pers.tile([64, HOT, 2, NQ], FP8, tag="ctxd")
        coef_rep = pers.tile([P, 1], F32, tag="coef_rep")
        bq_sb = pers.tile([P, HOT], F32, tag="bq_sb")
        bk_sb = pers.tile([P, HOT], F32, tag="bk_sb")
        bv_bf = pers.tile([1, H], BF16, tag="bv_bf")     # pre-scaled by WSC
        bo_bf = pers.tile([1, H], BF16, tag="bo_bf")     # pre-scaled by WSC
        ones1 = pers.tile([1, P], BF16, tag="ones1")
        ident64 = pers.tile([P, P], BF16, tag="ident64")
        wo_sb = pers.tile([64, HOT, 2, H], FP8, tag="wo_sb")

        ps_s = top.enter_context(tc.tile_pool(name="ps_s", bufs=2, space="PSUM"))
        ps_w = top.enter_context(tc.tile_pool(name="ps_w", bufs=2, space="PSUM"))
        ps_ctx = top.enter_context(tc.tile_pool(name="ps_ctx", bufs=2, space="PSUM"))

        # ---- constants ----
        nc.vector.memset(ones1[:], 1.0)
        make_identity(nc, ident64[:])
        nc.vector.tensor_scalar_mul(ident64[:], ident64[:], WSC)
        with ExitStack() as phproj:
            vhp = phproj.enter_context(tc.tile_pool(name="vhp", bufs=2))
            ebp = phproj.enter_context(tc.tile_pool(name="ebp", bufs=2))
            kqp = phproj.enter_context(tc.tile_pool(name="kqp", bufs=3))
            esp = phproj.enter_context(tc.tile_pool(name="esp", bufs=8))
            colp = phproj.enter_context(tc.tile_pool(name="colp", bufs=4))
            repp = phproj.enter_context(tc.tile_pool(name="repp", bufs=2))
            projmem = ExitStack()
            projp = projmem.enter_context(tc.tile_pool(name="projp", bufs=1))
            xTd = projp.tile([64, HOT, 2, NKV], FP8, tag="xTd")
            wv_sb = projp.tile([64, HOT, 2, H], FP8, tag="wv_sb")
            nc.sync.dma_start(xTd[:].rearrange("p c i s -> p (c i s)"), xTd_d)

            # per-pair streamed K/Q weight chunks -- first two pairs' DMAs
            # issued immediately so their projections can start right after
            # the xTd transfer lands
            kwp = projmem.enter_context(tc.tile_pool(name="kwp", bufs=2))

            def kq_dmas(pr):
                wkc = kwp.tile([64, HOT, 2, P], FP8, tag="wkc", name=f"wkc_{pr}")
                wqc = kwp.tile([64, HOT, 2, P], FP8, tag="wqc", name=f"wqc_{pr}")
                nc.sync.dma_start(
                    wkc[:],
                    w_d["wk"].rearrange("p (c i s) -> p c i s", c=HOT, i=2)[
                        :, :, :, pr * P : (pr + 1) * P
                    ],
                )
                nc.sync.dma_start(
                    wqc[:],
                    w_d["wq"].rearrange("p (c i s) -> p c i s", c=HOT, i=2)[
                        :, :, :, pr * P : (pr + 1) * P
                    ],
                )
                return wkc, wqc

            d0 = kq_dmas(0)
            d1 = kq_dmas(1)
            nc.sync.dma_start(
                wv_sb[:].rearrange("p c i s -> p (c i s)"), w_d["wv"]
            )
            nc.sync.dma_start(coef_rep[:], coef[None, :].to_broadcast((P, 1)))
            nc.sync.dma_start(bq_sb[:], bs["bq"].rearrange("(t p) -> p t", p=P))
            nc.sync.dma_start(bk_sb[:], bs["bk"].rearrange("(t p) -> p t", p=P))
            for src, dst in ((bs["bv"], bv_bf), (bs["bo"], bo_bf)):
                t = ebp.tile([1, H], F32, tag="ebstg")
                nc.sync.dma_start(t[:], src[None, :])
                # scale biases to match the host-scaled weights
                nc.vector.tensor_scalar_mul(dst[:], t[:], WSC)

            # vh: V' for all heads, [seq-part, kt, head, 64+1]
            vhs = {}

            # expB staging (DMA fp32 B^T chunks, exp with scale=coef)

            def expB_gen():
                for kt in range(KT):
                    stg = ebp.tile([P, NQ], F32, tag="ebstg")
                    nc.scalar.dma_start(stg[:], biasT[kt * P : (kt + 1) * P, :])
                    nc.scalar.activation(
                        expB[:, kt, :], stg[:], AF.Exp, scale=coef_rep[:, 0:1]
                    )
                    yield

            def vproj_gen(g):
                # V' for pairs 4g..4g+3 (heads 8g..8g+7): [128 seq, 512 dims]
                vh = vhp.tile([P, KT, 8, VW], BF16, tag="vh", name=f"vh_{g}")
                vhs[g] = vh
                nc.vector.memset(vh[:, :, :, HD : HD + 1], 1.0)
                for st in range(KT):
                    ps = ps_w.tile([P, 512], F32, tag="work")
                    for c in range(HOT):
                        nc.tensor.matmul(
                            ps[:],
                            xTd[:, c, :, st * P : (st + 1) * P],
                            wv_sb[:, c, :, g * 512 : (g + 1) * 512],
                            start=(c == 0),
                            stop=False,
                            perf_mode=DR,
                        )
                    nc.tensor.matmul(
                        ps[:],
                        ones1[:, 0:P],
                        bv_bf[:, g * 512 : (g + 1) * 512],
                        start=False,
                        stop=True,
                    )
                    nc.vector.tensor_scalar_mul(
                        vh[:, st, :, 0:HD],
                        ps[:].rearrange("p (a b) -> p a b", a=8),
                        IWSC,
                    )
                    yield

            def kqproj_gen(pr, wkc, wqc):
                kTp = kqp.tile([P, NKV], BF16, tag="kTp", name=f"kTp_{pr}")
                qTp = kqp.tile([P, NQ], BF16, tag="qTp", name=f"qTp_{pr}")
                kqs[pr] = (kTp, qTp)
                for sc in range(NKV // 512):
                    ps = ps_w.tile([P, 512], F32, tag="work")
                    for c in range(HOT):
                        nc.tensor.matmul(
                            ps[:],
                            wkc[:, c, :, :],
                            xTd[:, c, :, sc * 512 : (sc + 1) * 512],
                            start=(c == 0),
                            stop=(c == HOT - 1),
                            perf_mode=DR,
                        )
                    nc.vector.tensor_scalar(
                        kTp[:, sc * 512 : (sc + 1) * 512],
                        ps[:],
                        IWSC,
                        bk_sb[:, pr : pr + 1],
                        ALU.mult,
                        ALU.add,
                    )
                    yield
                for sc in range(NQ // 512):
                    ps = ps_w.tile([P, 512], F32, tag="work")
                    for c in range(HOT):
                        nc.tensor.matmul(
                            ps[:],
                            wqc[:, c, :, :],
                            xTd[:, c, :, sc * 512 : (sc + 1) * 512],
                            start=(c == 0),
                            stop=(c == HOT - 1),
                            perf_mode=DR,
                        )
                    nc.vector.tensor_scalar(
                        qTp[:, sc * 512 : (sc + 1) * 512],
                        ps[:],
                        IWSC,
                        bq_sb[:, pr : pr + 1],
                        ALU.mult,
                        ALU.add,
                    )
                    yield


            kqs = {}
            fillers = deque()

            def pump(n):
                k = 0
                while fillers and k < n:
                    try:
                        next(fillers[0])
                        k += 1
                    except StopIteration:
                        fillers.popleft()

            def drain(g):
                for _ in g:
                    pass
                try:
                    fillers.remove(g)
                except ValueError:
                    pass

            def attn_pass(pr, qc, per_kt=None):
                kTp, qTp = kqs[pr]
                vh = vhs[pr // 4]
                hb = (pr % 4) * 2  # head base within vh's 8-head group
                if True:
                    qsl = slice(qc * QTW, (qc + 1) * QTW)
                    cps = {
                        z: ps_ctx.tile(
                            [P, QTW], F32, tag="ctx", name=f"ctx_{pr}_{qc}_{z}"
                        )
                        for z in range(2)
                    }

                    def ctx_issue(kt, es):
                        for z in range(2):
                            nc.tensor.matmul(
                                cps[z][0:VW, :],
                                vh[:, kt, hb + z, :],
                                es[:, z, :],
                                start=(kt == 0),
                                stop=(kt == KT - 1),
                            )

                    pending_ctx = []  # ctx issued late so slow multiplies
                    for kt in range(KT):  # never block the in-order PE queue
                        es = esp.tile(
                            [P, 2, QTW], BF16, tag="es", name=f"es_{pr}_{qc}_{kt}"
                        )
                        sps = ps_s.tile(
                            [P, 2, QTW], F32, tag="s", name=f"s_{pr}_{qc}_{kt}"
                        )
                        for z in range(2):
                            r0 = z * HD
                            nc.tensor.matmul(
                                sps[:, z, :],
                                kTp[r0 : r0 + HD, kt * P : (kt + 1) * P],
                                qTp[r0 : r0 + HD, qsl],
                                start=True,
                                stop=True,
                            )
                        # one 1024-wide exp across both heads (2 PSUM banks)
                        nc.scalar.activation(es[:], sps[:], AF.Exp, scale=1.0 / 8.0)
                        for z in range(2):
                            eng = (
                                nc.gpsimd
                                if (z == 1 and kt < 4)
                                else nc.vector
                            )
                            eng.tensor_tensor(
                                es[:, z, :], es[:, z, :],
                                expB[:, kt, qsl], ALU.mult,
                            )
                        pending_ctx.append((kt, es))
                        if len(pending_ctx) > 6:
                            ctx_issue(*pending_ctx.pop(0))
                        if per_kt is not None:
                            per_kt(qc, kt)
                        pump(1)
                    for it in pending_ctx:
                        ctx_issue(*it)
                    fillers.appendleft(finisher(pr, qc, cps))

            def finisher(pr, qc, cps):
                # normalize + evacuate ctx^T in fp8 DoubleRow layout
                yield
                collr = {
                    z: colp.tile(
                        [1, QTW], BF16, tag="collr", name=f"collr_{pr}_{qc}_{z}"
                    )
                    for z in range(2)
                }
                with nc.allow_low_precision(reason="1/denom in bf16 is ample"):
                    for z in range(2):
                        nc.vector.reciprocal(
                            collr[z][:], cps[z][HD : HD + 1, :]
                        )
                yield
                rp = ps_w.tile([P, QTW], F32, tag="work", name=f"rp_{pr}_{qc}")
                for z in range(2):
                    nc.tensor.matmul(
                        rp[z * HD : (z + 1) * HD, :],
                        ones1[:, 0:HD],
                        collr[z][:],
                        start=True,
                        stop=True,
                    )
                rsb = repp.tile([P, QTW], BF16, tag="rsb", name=f"rsb_{pr}_{qc}")
                nc.vector.tensor_copy(rsb[:], rp[:])
                yield
                for z in range(2):
                    nc.vector.tensor_tensor(
                        ctxd[:, pr, z, qc * QTW : (qc + 1) * QTW],
                        cps[z][0:HD, :],
                        rsb[z * HD : (z + 1) * HD, :],
                        ALU.mult,
                    )

            ph3_state = {}

            def phase3_qt(qt, pspools):
                fin = ph3_state["fin"]
                gamma_rep = ph3_state["gamma_rep"]
                beta_rep = ph3_state["beta_rep"]
                xres = ph3_state["xres_all"][qt]
                y = fin.tile([P, H], F32, tag="y", name=f"y_{qt}")
                musum = fin.tile([P, NHC], F32, tag="musum")
                for hc in range(NHC):
                    pspool = pspools[hc % len(pspools)]
                    pso = pspool.tile(
                        [P, 512], F32,
                        tag="ctx" if pspool is ps_ctx else "work",
                    )
                    for pr in range(HOT):
                        nc.tensor.matmul(
                            pso[:],
                            ctxd[:, pr, :, qt * P : (qt + 1) * P],
                            wo_sb[:, pr, :, hc * CW : (hc + 1) * CW],
                            start=(pr == 0),
                            stop=False,
                            perf_mode=DR,
                        )
                    nc.tensor.matmul(
                        pso[:],
                        ones1[:, 0:P],
                        bo_bf[:, hc * CW : (hc + 1) * CW],
                        start=False,
                        stop=False,
                    )
                    # residual folded into the accumulation (64*xres)
                    nc.tensor.matmul(
                        pso[:],
                        ident64[:],
                        xres[:, hc * CW : (hc + 1) * CW],
                        start=False,
                        stop=True,
                    )
                    nc.scalar.activation(
                        y[:, hc * CW : (hc + 1) * CW],
                        pso[:],
                        AF.Copy,
                        scale=IWSC,
                        accum_out=musum[:, hc : hc + 1],
                    )
                    yield
                musumr = fin.tile([P, NHC], F32, tag="musumr")
                negmu = fin.tile([P, 1], F32, tag="negmu")
                nc.vector.tensor_scalar(
                    musumr[:], musum[:], -1.0 / H, 0.0, ALU.mult, ALU.add,
                    accum_out=negmu[:, 0:1],
                )
                sq = fin.tile([P, H], BF16, tag="sq")
                varsum = fin.tile([P, 1], F32, tag="varsum")
                nc.scalar.activation(
                    sq[:], y[:], AF.Square, bias=negmu[:, 0:1],
                    accum_out=varsum[:, 0:1],
                )
                vs2 = fin.tile([P, 1], F32, tag="vs2")
                nc.vector.tensor_scalar(
                    vs2[:], varsum[:], 1.0 / H, EPS, ALU.mult, ALU.add
                )
                vinv = fin.tile([P, 1], F32, tag="vinv")
                nc.vector.reciprocal(vinv[:], vs2[:])
                rstd = fin.tile([P, 1], F32, tag="rstd")
                nc.scalar.sqrt(rstd[:], vinv[:])
                yield
                t1 = fin.tile([P, H], F32, tag="t1", name=f"t1_{qt}")
                nc.vector.scalar_tensor_tensor(
                    t1[:], y[:], negmu[:, 0:1], gamma_rep[:], ALU.add, ALU.mult
                )
                ot = fin.tile([P, H], F32, tag="ot", name=f"ot_{qt}")
                nc.vector.scalar_tensor_tensor(
                    ot[:], t1[:], rstd[:, 0:1], beta_rep[:], ALU.mult, ALU.add
                )
                nc.sync.dma_start(out[qt * P : (qt + 1) * P, :], ot[:])

            # ---- schedule: V group 0 + first two pairs' K/Q upfront (their
            # weight DMAs issued before the bulk biasT stream), V group 1 and
            # later pairs' K/Q pumped into attention's PE slack per kt.
            gens = {0: kqproj_gen(0, *d0), 1: kqproj_gen(1, *d1)}
            drain(gens[0])
            drain(gens[1])
            gv0 = vproj_gen(0)
            for _ in range(3):
                next(gv0)
            fillers.append(gv0)
            geb = expB_gen()
            next(geb)

            def pace_expB(qc, kt):
                if qc == 0:
                    for _ in range(2 if kt >= 13 else 1):
                        try:
                            next(geb)
                        except StopIteration:
                            pass

            gv1 = None
            for pr in range(NPAIR - 1):
                if pr == 1:
                    nc.sync.dma_start(
                        wo_sb[:].rearrange("p c i s -> p (c i s)"), w_d["wo"]
                    )
                if pr + 2 < NPAIR:
                    dn = kq_dmas(pr + 2)
                    gens[pr + 2] = kqproj_gen(pr + 2, *dn)
                    fillers.append(gens[pr + 2])
                if pr == 1:
                    gv1 = vproj_gen(1)
                    fillers.append(gv1)
                if pr == 4 and gv1 is not None:
                    drain(gv1)
                drain(gens.pop(pr))
                pk = pace_expB if pr == 0 else None
                attn_pass(pr, 0, per_kt=pk)
                attn_pass(pr, 1, per_kt=pk)

            # ---- pair 7: qc1 first, then its qc0 pass overlapped with the
            # output projection + LayerNorm of the qc1 query range (qt 4..7).
            drain(gens.pop(7))
            projmem.close()   # free xTd/wv/weight-chunk SBUF for phase 3
            fin = phproj.enter_context(tc.tile_pool(name="fin", bufs=2))
            xrp = phproj.enter_context(tc.tile_pool(name="xrp", bufs=8))
            wof = phproj.enter_context(tc.tile_pool(name="wof", bufs=1))
            gamma_rep = wof.tile([P, H], F32, tag="gamma_rep")
            beta_rep = wof.tile([P, H], F32, tag="beta_rep")
            nc.gpsimd.dma_start(gamma_rep[:], gamma[None, :].to_broadcast((P, H)))
            nc.gpsimd.dma_start(beta_rep[:], beta[None, :].to_broadcast((P, H)))
            xres_all = []
            for qt in range(NQ // P):
                xr = xrp.tile([P, H], BF16, tag="xres", name=f"xres_{qt}")
                nc.gpsimd.dma_start(xr[:], xq[qt * P : (qt + 1) * P, :])
                xres_all.append(xr)
            ph3_state.update(
                fin=fin, gamma_rep=gamma_rep, beta_rep=beta_rep,
                xres_all=xres_all,
            )
            attn_pass(7, 1)
            for qt in range(4, 6):
                fillers.append(phase3_qt(qt, [ps_w]))
            attn_pass(7, 0)
            pump(10 ** 6)
            for qt in (6, 7, 0, 1, 2, 3):
                drain(phase3_qt(qt, [ps_ctx, ps_w]))

    if split:
        split_multi_waits(nc)
    return nc


_CACHE = {}


def _get_program(key=(2048, 1024, 1024, 16)):
    if key not in _CACHE:
        _CACHE[key] = build_program(*key)
    return _CACHE[key]


def _dr_pack(MT):
    """[H=1024 rows(h), C cols] -> fp8 [64, HOT=8, 2, C] with h = 128c+64i+p,
    flattened to [64, 8*2*C]."""
    C = MT.shape[1]
    t = MT.reshape(8, 2, 64, C).transpose(2, 0, 1, 3)
    return np.ascontiguousarray(t).astype(ml_dtypes.float8_e4m3).reshape(64, 8 * 2 * C)


def make_in_maps(hidden_states, bias_matrix_chunk, bias_coef,
                 Wq, bq, Wk, bk, Wv, bv, Wo, bo, ln_gamma, ln_beta,
                 B=4, S=2048):
    NQ = S // 2
    Wq = np.asarray(Wq, np.float32); Wk = np.asarray(Wk, np.float32)
    Wv = np.asarray(Wv, np.float32); Wo = np.asarray(Wo, np.float32)
    shared = {
        # pre-transposed (W.T: [h_in, out]) and pre-scaled by WSC, fp8 packed
        "wq": _dr_pack(Wq.T * WSC),
        "wk": _dr_pack(Wk.T * WSC),
        "wv": _dr_pack(Wv.T * WSC),
        "wo": _dr_pack(Wo.T * WSC),
        "bq": np.ascontiguousarray(bq, np.float32),
        "bk": np.ascontiguousarray(bk, np.float32),
        "bv": np.ascontiguousarray(bv, np.float32),
        "bo": np.ascontiguousarray(bo, np.float32),
        "gamma": np.ascontiguousarray(ln_gamma, np.float32),
        "beta": np.ascontiguousarray(ln_beta, np.float32),
        "coef": np.asarray(bias_coef, np.float32).reshape(1),
    }
    hs = np.asarray(hidden_states, np.float32)
    bm = np.asarray(bias_matrix_chunk, np.float32)
    in_maps = []
    for c in range(8):
        b, j = c // 2, c % 2
        m = dict(shared)
        if j == 0:
            perm_kv = hs[b]
            perm_bias = bm[:NQ, :]
        else:
            perm_kv = np.concatenate([hs[b, NQ:], hs[b, :NQ]], axis=0)
            perm_bias = np.concatenate([bm[NQ:, NQ:], bm[NQ:, :NQ]], axis=1)
        m["xTd"] = _dr_pack(perm_kv.T)          # X^T [h, seq] fp8 DR layout
        m["xq"] = np.ascontiguousarray(perm_kv[:NQ]).astype(ml_dtypes.bfloat16)
        m["biasT"] = np.ascontiguousarray(perm_bias.T, np.float32)
        in_maps.append(m)
    return in_maps


def kernel(hidden_states, bias_matrix_chunk, bias_coef,
           Wq, bq, Wk, bk, Wv, bv, Wo, bo, ln_gamma, ln_beta):
    B, S, H = 4, 2048, 1024
    NQ = S // 2
    nc = _get_program()
    in_maps = make_in_maps(
        hidden_states, bias_matrix_chunk, bias_coef,
        Wq, bq, Wk, bk, Wv, bv, Wo, bo, ln_gamma, ln_beta, B=B, S=S,
    )
    res = run_bass_kernel_spmd(nc, in_maps, core_ids=list(range(8)))
    outp = np.empty((B, S, H), np.float32)
    for c in range(8):
        b, j = c // 2, c % 2
        outp[b, j * NQ : (j + 1) * NQ] = res.results[c]["out"]
    return outp


# revision 77
# speedup vs baseline: 1.0060x; 1.0060x over previous
"""Trainium2 Bass kernel for CustomBertAttention (B=4, S=2048, H=1024, NH=16).

Sharding: 8 cores = (batch b in 0..3) x (query-half j in 0..1).
Each core computes attention for NQ=1024 query rows of one batch against the
full NKV=2048-key sequence of that batch.  The per-core KV sequence is
permuted host-side so the core's own query rows come FIRST (one SPMD program
for all cores); bias-matrix columns are permuted identically.

Precision/layout strategy (vs the straightforward version):
  * All projection operands (X^T, Wq/Wk/Wv/Wo^T) are packed HOST-SIDE into
    fp8e4m3 DoubleRow layout [64, c, 2, cols] (hidden h = 128c + 64i + p) so
    every projection matmul runs at 0.5 cycles/row -- and no on-device
    transposes or casts are needed at all.  Weights are pre-scaled by 2^6
    host-side (fp8e4m3 subnormal cutoff) and the 2^-6 is folded into the
    PSUM evacuation.
  * Scores/ctx stay bf16: K^T/Q^T per head-pair row-packed (2 heads at
    partitions 0:64 / 64:128), transposed scores S^T[k,q] per (z, qc-chunk)
    in one PSUM bank, exp on ScalarE (scale=1/8) into a per-kt es tile,
    multiplied by expB = exp(coef*B^T) (bf16; built once with the coef
    folded into the activation scale operand).
  * Denominators via the ones-augmented V' stationary (row 64 of the ctx
    PSUM); 1/denom computed by DVE reciprocal straight from PSUM, replicated
    across partitions with a ones-matmul, and applied IN the ctx evacuation
    multiply, which writes ctx^T directly in fp8 DoubleRow layout for the
    output projection.
  * Engine budget: ScalarE does only exp (+expB+LN tail), DVE does PSUM
    evacuations + most es*expB multiplies, GpSimd (Pool) takes a slice of
    the es multiplies (SBUF-only), PE does matmuls only.
"""

from collections import deque
from contextlib import ExitStack

import numpy as np
import ml_dtypes

import concourse.bass as bass
import concourse.mybir as mybir
import concourse.tile as tile
from concourse.bass_utils import run_bass_kernel_spmd
from concourse.masks import make_identity

F32 = mybir.dt.float32
BF16 = mybir.dt.bfloat16
FP8 = mybir.dt.float8e4
AF = mybir.ActivationFunctionType
AX = mybir.AxisListType
ALU = mybir.AluOpType
DR = mybir.MatmulPerfMode.DoubleRow

P = 128
EPS = 1e-12
I32 = mybir.dt.int32
FEXP_A = float(2 ** 23 / np.log(2)) / 8.0   # fast-exp scale (folds 1/8)
FEXP_B = float(127 * 2 ** 23 - 366000)      # fast-exp bias (error centering)
WSC = 64.0      # host-side weight pre-scale (2^6), undone in evacuations
IWSC = 1.0 / WSC


def split_multi_waits(nc):
    """Pinned walrus supports only ONE sync-wait per instruction; split extras
    onto preceding same-engine NoOps."""
    n_split = 0
    for fn in nc.m.functions:
        for blk in fn.blocks:
            new_insts = []
            for inst in blk.instructions:
                si = inst.sync_info
                if si is not None and si.on_wait and len(si.on_wait) > 1:
                    waits = list(si.on_wait)
                    for w in waits[:-1]:
                        nop = mybir.InstNoOp(
                            name=f"{inst.name}-wsplit{n_split}",
                            engine=inst.engine,
                        )
                        nop.sync_info = mybir.SyncInfo(on_wait=[w], on_update=[])
                        new_insts.append(nop)
                        n_split += 1
                    inst.sync_info = mybir.SyncInfo(
                        on_wait=[waits[-1]], on_update=list(si.on_update)
                    )
                new_insts.append(inst)
            blk.instructions = new_insts
    return n_split


def build_program(NKV=2048, NQ=1024, H=1024, NH=16, split=True):
    HD = H // NH
    assert HD == 64
    KT = NKV // P           # key seq tiles (16)
    HOT = H // P            # 128-wide hidden chunks (8) = head pairs
    QTW = 512               # q span per score matmul
    NQC = NQ // QTW         # 2
    NPAIR = NH // 2         # 8
    VW = HD + 1             # V' width per head (64 + ones col)
    CW = 512
    NHC = H // CW

    nc = bass.Bass("TRN2", target_bir_lowering=False, debug=False)

    # fp8 DoubleRow-packed operands: [64, c, i, cols] flat as [64, HOT*2*cols]
    xTd_d = nc.dram_tensor("xTd", [64, HOT * 2 * NKV], FP8, kind="ExternalInput").ap()
    w_d = {}
    for wname in ("wk", "wq", "wv", "wo"):
        w_d[wname] = nc.dram_tensor(wname, [64, HOT * 2 * H], FP8, kind="ExternalInput").ap()
    xq = nc.dram_tensor("xq", [NQ, H], BF16, kind="ExternalInput").ap()
    biasT = nc.dram_tensor("biasT", [NKV, NQ], F32, kind="ExternalInput").ap()
    bs = {}
    for bname in ("bq", "bk", "bv", "bo"):
        bs[bname] = nc.dram_tensor(bname, [H], F32, kind="ExternalInput").ap()
    gamma = nc.dram_tensor("gamma", [H], F32, kind="ExternalInput").ap()
    beta = nc.dram_tensor("beta", [H], F32, kind="ExternalInput").ap()
    coef = nc.dram_tensor("coef", [1], F32, kind="ExternalInput").ap()
    out = nc.dram_tensor("out", [NQ, H], F32, kind="ExternalOutput").ap()

    with tile.TileContext(nc) as tc, ExitStack() as top:
        pers = top.enter_context(tc.tile_pool(name="pers", bufs=1))
        expB = pers.tile([P, KT, NQ], BF16, tag="expB")
        ctxd = pers.tile([64, HOT, 2, NQ], FP8, tag="ctxd")
        coef_rep = pers.tile([P, 1], F32, tag="coef_rep")
        bq_sb = pers.tile([P, HOT], F32, tag="bq_sb")
        bk_sb = pers.tile([P, HOT], F32, tag="bk_sb")
        bv_bf = pers.tile([1, H], BF16, tag="bv_bf")     # pre-scaled by WSC
        bo_bf = pers.tile([1, H], BF16, tag="bo_bf")     # pre-scaled by WSC
        ones1 = pers.tile([1, P], BF16, tag="ones1")
        ident64 = pers.tile([P, P], BF16, tag="ident64")
        wo_sb = pers.tile([64, HOT, 2, H], FP8, tag="wo_sb")

        ps_s = top.enter_context(tc.tile_pool(name="ps_s", bufs=2, space="PSUM"))
        ps_w = top.enter_context(tc.tile_pool(name="ps_w", bufs=2, space="PSUM"))
        ps_ctx = top.enter_context(tc.tile_pool(name="ps_ctx", bufs=2, space="PSUM"))

        # ---- constants ----
        nc.vector.memset(ones1[:], 1.0)
        make_identity(nc, ident64[:])
        nc.vector.tensor_scalar_mul(ident64[:], ident64[:], WSC)
        with ExitStack() as phproj:
            vhp = phproj.enter_context(tc.tile_pool(name="vhp", bufs=2))
            ebp = phproj.enter_context(tc.tile_pool(name="ebp", bufs=3))
            kqp = phproj.enter_context(tc.tile_pool(name="kqp", bufs=3))
            esp = phproj.enter_context(tc.tile_pool(name="esp", bufs=7))
            colp = phproj.enter_context(tc.tile_pool(name="colp", bufs=4))
            repp = phproj.enter_context(tc.tile_pool(name="repp", bufs=2))
            projmem = ExitStack()
            projp = projmem.enter_context(tc.tile_pool(name="projp", bufs=1))
            xTd = projp.tile([64, HOT, 2, NKV], FP8, tag="xTd")
            wv_sb = projp.tile([64, HOT, 2, H], FP8, tag="wv_sb")
            nc.sync.dma_start(xTd[:].rearrange("p c i s -> p (c i s)"), xTd_d)

            # per-pair streamed K/Q weight chunks -- first two pairs' DMAs
            # issued immediately so their projections can start right after
            # the xTd transfer lands
            kwp = projmem.enter_context(tc.tile_pool(name="kwp", bufs=2))

            def kq_dmas(pr):
                wkc = kwp.tile([64, HOT, 2, P], FP8, tag="wkc", name=f"wkc_{pr}")
                wqc = kwp.tile([64, HOT, 2, P], FP8, tag="wqc", name=f"wqc_{pr}")
                nc.sync.dma_start(
                    wkc[:],
                    w_d["wk"].rearrange("p (c i s) -> p c i s", c=HOT, i=2)[
                        :, :, :, pr * P : (pr + 1) * P
                    ],
                )
                nc.sync.dma_start(
                    wqc[:],
                    w_d["wq"].rearrange("p (c i s) -> p c i s", c=HOT, i=2)[
                        :, :, :, pr * P : (pr + 1) * P
                    ],
                )
                return wkc, wqc

            d0 = kq_dmas(0)
            d1 = kq_dmas(1)
            nc.sync.dma_start(
                wv_sb[:].rearrange("p c i s -> p (c i s)"), w_d["wv"]
            )
            nc.sync.dma_start(coef_rep[:], coef[None, :].to_broadcast((P, 1)))
            nc.sync.dma_start(bq_sb[:], bs["bq"].rearrange("(t p) -> p t", p=P))
            nc.sync.dma_start(bk_sb[:], bs["bk"].rearrange("(t p) -> p t", p=P))
            for src, dst in ((bs["bv"], bv_bf), (bs["bo"], bo_bf)):
                t = ebp.tile([1, H], F32, tag="ebstg")
                nc.sync.dma_start(t[:], src[None, :])
                # scale biases to match the host-scaled weights
                nc.vector.tensor_scalar_mul(dst[:], t[:], WSC)

            # vh: V' for all heads, [seq-part, kt, head, 64+1]
            vhs = {}

            # expB staging (DMA fp32 B^T chunks, exp with scale=coef)

            def expB_gen():
                for kt in range(KT):
                    stg = ebp.tile([P, NQ], F32, tag="ebstg")
                    nc.scalar.dma_start(stg[:], biasT[kt * P : (kt + 1) * P, :])
                    nc.scalar.activation(
                        expB[:, kt, :], stg[:], AF.Exp, scale=coef_rep[:, 0:1]
                    )
                    yield

            def vproj_gen(g):
                # V' for pairs 4g..4g+3 (heads 8g..8g+7): [128 seq, 512 dims]
                vh = vhp.tile([P, KT, 8, VW], BF16, tag="vh", name=f"vh_{g}")
                vhs[g] = vh
                nc.vector.memset(vh[:, :, :, HD : HD + 1], 1.0)
                for st in range(KT):
                    ps = ps_w.tile([P, 512], F32, tag="work")
                    for c in range(HOT):
                        nc.tensor.matmul(
                            ps[:],
                            xTd[:, c, :, st * P : (st + 1) * P],
                            wv_sb[:, c, :, g * 512 : (g + 1) * 512],
                            start=(c == 0),
                            stop=False,
                            perf_mode=DR,
                        )
                    nc.tensor.matmul(
                        ps[:],
                        ones1[:, 0:P],
                        bv_bf[:, g * 512 : (g + 1) * 512],
                        start=False,
                        stop=True,
                    )
                    nc.vector.tensor_scalar_mul(
                        vh[:, st, :, 0:HD],
                        ps[:].rearrange("p (a b) -> p a b", a=8),
                        IWSC,
                    )
                    yield

            def kqproj_gen(pr, wkc, wqc):
                kTp = kqp.tile([P, NKV], BF16, tag="kTp", name=f"kTp_{pr}")
                qTp = kqp.tile([P, NQ], BF16, tag="qTp", name=f"qTp_{pr}")
                kqs[pr] = (kTp, qTp)
                for sc in range(NKV // 512):
                    ps = ps_w.tile([P, 512], F32, tag="work")
                    for c in range(HOT):
                        nc.tensor.matmul(
                            ps[:],
                            wkc[:, c, :, :],
                            xTd[:, c, :, sc * 512 : (sc + 1) * 512],
                            start=(c == 0),
                            stop=(c == HOT - 1),
                            perf_mode=DR,
                        )
                    nc.vector.tensor_scalar(
                        kTp[:, sc * 512 : (sc + 1) * 512],
                        ps[:],
                        IWSC,
                        bk_sb[:, pr : pr + 1],
                        ALU.mult,
                        ALU.add,
                    )
                    yield
                for sc in range(NQ // 512):
                    ps = ps_w.tile([P, 512], F32, tag="work")
                    for c in range(HOT):
                        nc.tensor.matmul(
                            ps[:],
                            wqc[:, c, :, :],
                            xTd[:, c, :, sc * 512 : (sc + 1) * 512],
                            start=(c == 0),
                            stop=(c == HOT - 1),
                            perf_mode=DR,
                        )
                    nc.vector.tensor_scalar(
                        qTp[:, sc * 512 : (sc + 1) * 512],
                        ps[:],
                        IWSC,
                        bq_sb[:, pr : pr + 1],
                        ALU.mult,
                        ALU.add,
                    )
                    yield


            kqs = {}
            fillers = deque()

            def pump(n):
                k = 0
                while fillers and k < n:
                    try:
                        next(fillers[0])
                        k += 1
                    except StopIteration:
                        fillers.popleft()

            def drain(g):
                for _ in g:
                    pass
                try:
                    fillers.remove(g)
                except ValueError:
                    pass

            def attn_pass(pr, qc, per_kt=None):
                kTp, qTp = kqs[pr]
                vh = vhs[pr // 4]
                hb = (pr % 4) * 2  # head base within vh's 8-head group
                if True:
                    qsl = slice(qc * QTW, (qc + 1) * QTW)
                    cps = {
                        z: ps_ctx.tile(
                            [P, QTW], F32, tag="ctx", name=f"ctx_{pr}_{qc}_{z}"
                        )
                        for z in range(2)
                    }

                    def ctx_issue(kt, es):
                        for z in range(2):
                            nc.tensor.matmul(
                                cps[z][0:VW, :],
                                vh[:, kt, hb + z, :],
                                es[:, z, :],
                                start=(kt == 0),
                                stop=(kt == KT - 1),
                            )

                    pending_ctx = []  # ctx issued late so slow multiplies
                    for kt in range(KT):  # never block the in-order PE queue
                        es = esp.tile(
                            [P, 2, QTW], BF16, tag="es", name=f"es_{pr}_{qc}_{kt}"
                        )
                        sps = ps_s.tile(
                            [P, 2, QTW], F32, tag="s", name=f"s_{pr}_{qc}_{kt}"
                        )
                        for z in range(2):
                            r0 = z * HD
                            nc.tensor.matmul(
                                sps[:, z, :],
                                kTp[r0 : r0 + HD, kt * P : (kt + 1) * P],
                                qTp[r0 : r0 + HD, qsl],
                                start=True,
                                stop=True,
                            )
                        # one 1024-wide exp across both heads (2 PSUM banks)
                        nc.scalar.activation(es[:], sps[:], AF.Exp, scale=1.0 / 8.0)
                        for z in range(2):
                            eng = (
                                nc.gpsimd
                                if (z == 1 and kt < 4)
                                else nc.vector
                            )
                            eng.tensor_tensor(
                                es[:, z, :], es[:, z, :],
                                expB[:, kt, qsl], ALU.mult,
                            )
                        pending_ctx.append((kt, es))
                        if len(pending_ctx) > 6:
                            ctx_issue(*pending_ctx.pop(0))
                        if per_kt is not None:
                            per_kt(qc, kt)
                        pump(1)
                    for it in pending_ctx:
                        ctx_issue(*it)
                    fillers.appendleft(finisher(pr, qc, cps))

            def finisher(pr, qc, cps):
                # normalize + evacuate ctx^T in fp8 DoubleRow layout
                yield
                collr = {
                    z: colp.tile(
                        [1, QTW], BF16, tag="collr", name=f"collr_{pr}_{qc}_{z}"
                    )
                    for z in range(2)
                }
                with nc.allow_low_precision(reason="1/denom in bf16 is ample"):
                    for z in range(2):
                        nc.vector.reciprocal(
                            collr[z][:], cps[z][HD : HD + 1, :]
                        )
                yield
                rp = ps_w.tile([P, QTW], F32, tag="work", name=f"rp_{pr}_{qc}")
                for z in range(2):
                    nc.tensor.matmul(
                        rp[z * HD : (z + 1) * HD, :],
                        ones1[:, 0:HD],
                        collr[z][:],
                        start=True,
                        stop=True,
                    )
                rsb = repp.tile([P, QTW], BF16, tag="rsb", name=f"rsb_{pr}_{qc}")
                nc.vector.tensor_copy(rsb[:], rp[:])
                yield
                for z in range(2):
                    nc.vector.tensor_tensor(
                        ctxd[:, pr, z, qc * QTW : (qc + 1) * QTW],
                        cps[z][0:HD, :],
                        rsb[z * HD : (z + 1) * HD, :],
                        ALU.mult,
                    )

            ph3_state = {}

            def phase3_qt(qt, pspools):
                fin = ph3_state["fin"]
                gamma_rep = ph3_state["gamma_rep"]
                beta_rep = ph3_state["beta_rep"]
                xres = ph3_state["xres_all"][qt]
                y = fin.tile([P, H], F32, tag="y", name=f"y_{qt}")
                musum = fin.tile([P, NHC], F32, tag="musum")
                for hc in range(NHC):
                    pspool = pspools[hc % len(pspools)]
                    pso = pspool.tile(
                        [P, 512], F32,
                        tag="ctx" if pspool is ps_ctx else "work",
                    )
                    for pr in range(HOT):
                        nc.tensor.matmul(
                            pso[:],
                            ctxd[:, pr, :, qt * P : (qt + 1) * P],
                            wo_sb[:, pr, :, hc * CW : (hc + 1) * CW],
                            start=(pr == 0),
                            stop=False,
                            perf_mode=DR,
                        )
                    nc.tensor.matmul(
                        pso[:],
                        ones1[:, 0:P],
                        bo_bf[:, hc * CW : (hc + 1) * CW],
                        start=False,
                        stop=False,
                    )
                    # residual folded into the accumulation (64*xres)
                    nc.tensor.matmul(
                        pso[:],
                        ident64[:],
                        xres[:, hc * CW : (hc + 1) * CW],
                        start=False,
                        stop=True,
                    )
                    nc.scalar.activation(
                        y[:, hc * CW : (hc + 1) * CW],
                        pso[:],
                        AF.Copy,
                        scale=IWSC,
                        accum_out=musum[:, hc : hc + 1],
                    )
                    yield
                musumr = fin.tile([P, NHC], F32, tag="musumr")
                negmu = fin.tile([P, 1], F32, tag="negmu")
                nc.vector.tensor_scalar(
                    musumr[:], musum[:], -1.0 / H, 0.0, ALU.mult, ALU.add,
                    accum_out=negmu[:, 0:1],
                )
                sq = fin.tile([P, H], BF16, tag="sq")
                varsum = fin.tile([P, 1], F32, tag="varsum")
                nc.scalar.activation(
                    sq[:], y[:], AF.Square, bias=negmu[:, 0:1],
                    accum_out=varsum[:, 0:1],
                )
                vs2 = fin.tile([P, 1], F32, tag="vs2")
                nc.vector.tensor_scalar(
                    vs2[:], varsum[:], 1.0 / H, EPS, ALU.mult, ALU.add
                )
                vinv = fin.tile([P, 1], F32, tag="vinv")
                nc.vector.reciprocal(vinv[:], vs2[:])
                rstd = fin.tile([P, 1], F32, tag="rstd")
                nc.scalar.sqrt(rstd[:], vinv[:])
                yield
                t1 = fin.tile([P, H], F32, tag="t1", name=f"t1_{qt}")
                nc.vector.scalar_tensor_tensor(
                    t1[:], y[:], negmu[:, 0:1], gamma_rep[:], ALU.add, ALU.mult
                )
                ot = fin.tile([P, H], F32, tag="ot", name=f"ot_{qt}")
                nc.vector.scalar_tensor_tensor(
                    ot[:], t1[:], rstd[:, 0:1], beta_rep[:], ALU.mult, ALU.add
                )
                nc.sync.dma_start(out[qt * P : (qt + 1) * P, :], ot[:])

            # ---- schedule: V group 0 + first two pairs' K/Q upfront (their
            # weight DMAs issued before the bulk biasT stream), V group 1 and
            # later pairs' K/Q pumped into attention's PE slack per kt.
            gens = {0: kqproj_gen(0, *d0), 1: kqproj_gen(1, *d1)}
            drain(gens[0])
            drain(gens[1])
            gv0 = vproj_gen(0)
            for _ in range(3):
                next(gv0)
            fillers.append(gv0)
            geb = expB_gen()
            next(geb)

            def pace_expB(qc, kt):
                if qc == 0:
                    for _ in range(2 if kt >= 13 else 1):
                        try:
                            next(geb)
                        except StopIteration:
                            pass

            gv1 = None
            for pr in range(NPAIR - 1):
                if pr == 1:
                    nc.sync.dma_start(
                        wo_sb[:].rearrange("p c i s -> p (c i s)"), w_d["wo"]
                    )
                if pr + 2 < NPAIR:
                    dn = kq_dmas(pr + 2)
                    gens[pr + 2] = kqproj_gen(pr + 2, *dn)
                    fillers.append(gens[pr + 2])
                if pr == 1:
                    gv1 = vproj_gen(1)
                    fillers.append(gv1)
                if pr == 4 and gv1 is not None:
                    drain(gv1)
                drain(gens.pop(pr))
                pk = pace_expB if pr == 0 else None
                attn_pass(pr, 0, per_kt=pk)
                attn_pass(pr, 1, per_kt=pk)

            # ---- pair 7: qc1 first, then its qc0 pass overlapped with the
            # output projection + LayerNorm of the qc1 query range (qt 4..7).
            drain(gens.pop(7))
            projmem.close()   # free xTd/wv/weight-chunk SBUF for phase 3
            fin = phproj.enter_context(tc.tile_pool(name="fin", bufs=2))
            xrp = phproj.enter_context(tc.tile_pool(name="xrp", bufs=8))
            wof = phproj.enter_context(tc.tile_pool(name="wof", bufs=1))
            gamma_rep = wof.tile([P, H], F32, tag="gamma_rep")
            beta_rep = wof.tile([P, H], F32, tag="beta_rep")
            nc.gpsimd.dma_start(gamma_rep[:], gamma[None, :].to_broadcast((P, H)))
            nc.gpsimd.dma_start(beta_rep[:], beta[None, :].to_broadcast((P, H)))
            xres_all = []
            for qt in range(NQ // P):
                xr = xrp.tile([P, H], BF16, tag="xres", name=f"xres_{qt}")
                nc.gpsimd.dma_start(xr[:], xq[qt * P : (qt + 1) * P, :])
                xres_all.append(xr)
            ph3_state.update(
                fin=fin, gamma_rep=gamma_rep, beta_rep=beta_rep,
                xres_all=xres_all,
            )
            attn_pass(7, 1)
            for qt in range(4, 6):
                fillers.append(phase3_qt(qt, [ps_w]))
            attn_pass(7, 0)
            pump(10 ** 6)
            for qt in (6, 7, 0, 1, 2, 3):
                drain(phase3_qt(qt, [ps_ctx, ps_w]))

    if split:
        split_multi_waits(nc)
    return nc


_CACHE = {}


def _get_program(key=(2048, 1024, 1024, 16)):
    if key not in _CACHE:
        _CACHE[key] = build_program(*key)
    return _CACHE[key]


def _dr_pack(MT):
    """[H=1024 rows(h), C cols] -> fp8 [64, HOT=8, 2, C] with h = 128c+64i+p,
    flattened to [64, 8*2*C]."""
    C = MT.shape[1]
    t = MT.reshape(8, 2, 64, C).transpose(2, 0, 1, 3)
    return np.ascontiguousarray(t).astype(ml_dtypes.float8_e4m3).reshape(64, 8 * 2 * C)


def make_in_maps(hidden_states, bias_matrix_chunk, bias_coef,
                 Wq, bq, Wk, bk, Wv, bv, Wo, bo, ln_gamma, ln_beta,
                 B=4, S=2048):
    NQ = S // 2
    Wq = np.asarray(Wq, np.float32); Wk = np.asarray(Wk, np.float32)
    Wv = np.asarray(Wv, np.float32); Wo = np.asarray(Wo, np.float32)
    shared = {
        # pre-transposed (W.T: [h_in, out]) and pre-scaled by WSC, fp8 packed
        "wq": _dr_pack(Wq.T * WSC),
        "wk": _dr_pack(Wk.T * WSC),
        "wv": _dr_pack(Wv.T * WSC),
        "wo": _dr_pack(Wo.T * WSC),
        "bq": np.ascontiguousarray(bq, np.float32),
        "bk": np.ascontiguousarray(bk, np.float32),
        "bv": np.ascontiguousarray(bv, np.float32),
        "bo": np.ascontiguousarray(bo, np.float32),
        "gamma": np.ascontiguousarray(ln_gamma, np.float32),
        "beta": np.ascontiguousarray(ln_beta, np.float32),
        "coef": np.asarray(bias_coef, np.float32).reshape(1),
    }
    hs = np.asarray(hidden_states, np.float32)
    bm = np.asarray(bias_matrix_chunk, np.float32)
    in_maps = []
    for c in range(8):
        b, j = c // 2, c % 2
        m = dict(shared)
        if j == 0:
            perm_kv = hs[b]
            perm_bias = bm[:NQ, :]
        else:
            perm_kv = np.concatenate([hs[b, NQ:], hs[b, :NQ]], axis=0)
            perm_bias = np.concatenate([bm[NQ:, NQ:], bm[NQ:, :NQ]], axis=1)
        m["xTd"] = _dr_pack(perm_kv.T)          # X^T [h, seq] fp8 DR layout
        m["xq"] = np.ascontiguousarray(perm_kv[:NQ]).astype(ml_dtypes.bfloat16)
        m["biasT"] = np.ascontiguousarray(perm_bias.T, np.float32)
        in_maps.append(m)
    return in_maps


def kernel(hidden_states, bias_matrix_chunk, bias_coef,
           Wq, bq, Wk, bk, Wv, bv, Wo, bo, ln_gamma, ln_beta):
    B, S, H = 4, 2048, 1024
    NQ = S // 2
    nc = _get_program()
    in_maps = make_in_maps(
        hidden_states, bias_matrix_chunk, bias_coef,
        Wq, bq, Wk, bk, Wv, bv, Wo, bo, ln_gamma, ln_beta, B=B, S=S,
    )
    res = run_bass_kernel_spmd(nc, in_maps, core_ids=list(range(8)))
    outp = np.empty((B, S, H), np.float32)
    for c in range(8):
        b, j = c // 2, c % 2
        outp[b, j * NQ : (j + 1) * NQ] = res.results[c]["out"]
    return outp
